# revision 36
# baseline (speedup 1.0000x reference)
"""Trainium2 Bass kernel for nn_MultiHeadAttention_77412490543447 (v2).

reference:
  qkv = x @ W_qkv + b_qkv -> q,k,v  (B,H,S,D)
  scores = scale*(q k^T) + scale*einsum('xyc,bhxc->bhxy', pe, q)  [no softmax]
  out = (scores @ v) @ W_out + b_out

No softmax => (q k^T) @ v == q @ (k^T v): the qk path collapses to per-head
64x64 matrices (AllReduce'd across cores); only the pe term needs S*S work,
and it is ~2% of the output magnitude, so the whole pe path runs in fp8.

Sharding: query-position (x) blocks of 128 per core, 8 cores.  Each core
projects q/k/v for its OWN 512 tokens (N=512 matmuls), AllGathers v in fp8,
streams its pe slice (fp8) from DRAM, computes pe-scores with K=64
row-alternating matmuls (fp8 FWL weight loads), keeps scores fp8 in SBUF,
and accumulates attn = q@(k^T v) + scores^T_stationary @ v in PSUM [x, feat].
A DVE stream-transpose grid converts [x, feat] -> [feat, x] for the output
projection.
"""

import os
import numpy as np
import ml_dtypes

import concourse.bass as bass
import concourse.bacc as bacc
import concourse.mybir as mybir
import concourse.tile as tile
from concourse.bass_utils import run_bass_kernel_spmd

BF = mybir.dt.bfloat16
F8 = mybir.dt.float8e4
F32 = mybir.dt.float32
ADD = mybir.AluOpType.add
BYPASS = mybir.AluOpType.bypass

B, S, E = 4, 1024, 1024
H, D = 16, 64
HP = H // 2               # head pairs
NCORES = 8
XB = S // NCORES          # 128 query positions per core
OWN = B * XB              # 512 own tokens
KC = E // 128             # 8 contraction chunks
YC = S // 128             # 8 key-position chunks
XP = XB // 2              # 64 x-pairs

_compiled = None
KPHASES = int(os.environ.get('KPHASES', '9'))
NOAG = int(os.environ.get('NOAG', '0'))
NOAR = int(os.environ.get('NOAR', '0'))


def build_kernel():
    nc = bacc.Bacc(None, target_bir_lowering=False)

    xTo = nc.dram_tensor("xTo", [E, OWN], BF, kind="ExternalInput")
    wq = nc.dram_tensor("wq", [E, E], BF, kind="ExternalInput")
    wk = nc.dram_tensor("wk", [E, E], BF, kind="ExternalInput")
    wv = nc.dram_tensor("wv", [E, E], BF, kind="ExternalInput")
    wo = nc.dram_tensor("wo", [E, E], BF, kind="ExternalInput")
    pet = nc.dram_tensor("pet", [XP, 128, S], F8, kind="ExternalInput")
    bq = nc.dram_tensor("bq", [1, E], BF, kind="ExternalInput")
    bk = nc.dram_tensor("bk", [1, E], BF, kind="ExternalInput")
    bv = nc.dram_tensor("bv", [1, E], BF, kind="ExternalInput")
    bo = nc.dram_tensor("bo", [1, E], BF, kind="ExternalInput")
    out = nc.dram_tensor("out", [OWN, E], BF, kind="ExternalOutput")

    with tile.TileContext(nc) as tc:
        with (
            tc.tile_pool(name="dram", bufs=1, space="DRAM") as dram,
            tc.tile_pool(name="const", bufs=1) as const,
            tc.tile_pool(name="res", bufs=1) as res,
            tc.tile_pool(name="stage", bufs=2) as stage,
        ):
            # collective bounce buffers (DRAM)
            ag_in = dram.tile([128, B, E], F8)            # own v shard (fp8)
            ag_out = dram.tile([NCORES, 128, B, E], F8, addr_space="Shared")
            ar_in = dram.tile([128, HP * B * D], BF)      # local k^T v
            ar_out = dram.tile([128, HP * B * D], BF, addr_space="Shared")

            ones = const.tile([1, 512], BF)
            nc.vector.memset(ones[:], 1.0)
            bq_sb = const.tile([1, E], BF, tag="bq")
            bk_sb = const.tile([1, E], BF, tag="bk")
            bv_sb = const.tile([1, E], BF, tag="bv")
            bo_sb = const.tile([1, E], BF, tag="bo")
            nc.gpsimd.dma_start(bq_sb[:], bq[:])
            nc.gpsimd.dma_start(bk_sb[:], bk[:])
            nc.gpsimd.dma_start(bv_sb[:], bv[:])
            nc.gpsimd.dma_start(bo_sb[:], bo[:])

            # resident tensors
            k_own = res.tile([128, B, E], BF, tag="k_own")
            v_own = res.tile([128, B, E], BF, tag="v_own")
            vq8 = res.tile([128, B, E], F8, tag="vq8")
            # qB: [ (h%2)*64+c, hp, b, x ] bf16  (attn1 lhsT)
            qB = res.tile([128, HP, B, XB], BF, tag="qB")
            # qP8: [ (x//64)*64+c, x%64, bh ] fp8  (pe-score rhs); bh = h*B+b
            qP8 = res.tile([128, XP, H * B], F8, tag="qP8")
            # M (k^T v): [ (h%2)*64+c, hp, b, d ]
            M_sb = res.tile([128, HP, B, D], BF, tag="M_sb")
            # block-diagonal M for K=128 attn1: [ c-pair, hp, b, d-pair ]
            M2blk = res.tile([128, HP, B, 128], BF, tag="M2blk")
            # scores: [ y%128, yc, x, bh ] fp8 (bh innermost: contiguous evict)
            S_sb = res.tile([128, YC, XB, H * B], F8, tag="S_sb")
            # attnT: [ f%128 = (h%2)*64+d, f//128 = hp, b, x ] bf16
            attnT = res.tile([128, KC, B, XB], BF, tag="attnT")

            xTo_sb = res.tile([128, KC, OWN], BF, tag="xTo")
            for kc in range(KC):
                nc.gpsimd.dma_start(xTo_sb[:, kc, :], xTo[kc * 128:(kc + 1) * 128, :])

            # ---------------- projections + k^T v ----------------
            with (
                tc.tile_pool(name="wpool", bufs=3) as wpool,
                tc.tile_pool(name="psP", bufs=6, space="PSUM") as psP,
                tc.tile_pool(name="psM", bufs=2, space="PSUM") as psM,
            ):
                wk_sb = wpool.tile([128, KC, E], BF, tag="w", name="wk_sb")
                wv_sb = wpool.tile([128, KC, E], BF, tag="w", name="wv_sb")
                wq_sb = wpool.tile([128, KC, E], BF, tag="w", name="wq_sb")
                for kc in range(KC):
                    nc.sync.dma_start(wk_sb[:, kc, :], wk[kc * 128:(kc + 1) * 128, :])
                for kc in range(KC):
                    nc.sync.dma_start(wv_sb[:, kc, :], wv[kc * 128:(kc + 1) * 128, :])
                for kc in range(KC):
                    nc.sync.dma_start(wq_sb[:, kc, :], wq[kc * 128:(kc + 1) * 128, :])
                # q projection (feature-major: [feat, tok]), scale pre-folded
                for hp in range(HP if KPHASES >= 3 else 0):
                    ps = psP.tile([128, 512], F32, tag="ps")
                    for kc in range(KC):
                        nc.tensor.matmul(
                            ps[:],
                            wq_sb[:, kc, hp * 128:(hp + 1) * 128],
                            xTo_sb[:, kc, :],
                            start=(kc == 0), stop=False,
                        )
                    nc.tensor.matmul(
                        ps[:], bq_sb[:, hp * 128:(hp + 1) * 128], ones[:],
                        start=False, stop=True)
                    # qB: [(h%2)*64+c, hp, b, x] <- ps [(h%2)*64+c, (b x)]
                    qb_eng = nc.scalar if hp % 2 == 0 else nc.vector
                    qb_copy = (nc.scalar.copy if hp % 2 == 0
                               else nc.vector.tensor_copy)
                    qb_copy(
                        qB[:, hp, :, :],
                        ps[:].rearrange("p (b x) -> p b x", b=B))
                    # qP8: [(x//64)*64+c, x%64, h*B+b] <- ps[par*64+c, (b, x)]
                    for par in range(2):
                        h = 2 * hp + par
                        for xh in range(2):
                            src = ps[par * 64:(par + 1) * 64, :].rearrange(
                                "c (b xh p) -> c xh b p", b=B, xh=2)[
                                :, xh, :, :]
                            dst = qP8[xh * 64:(xh + 1) * 64, :, :].rearrange(
                                "c p (h b) -> c h b p", h=H)[:, h, :, :]
                            if hp % 2 == 0:
                                nc.vector.tensor_copy(dst, src)
                            else:
                                nc.scalar.copy(dst, src)

                # k+v projections fused: share the xTo stationary across the
                # 4 output halves per contraction chunk
                for b in range(B if KPHASES >= 1 else 0):
                    pk = [psP.tile([128, 512], F32, tag="ps", name=f"pk{b}_{i}")
                          for i in range(2)]
                    pv = [psP.tile([128, 512], F32, tag="ps", name=f"pv{b}_{i}")
                          for i in range(2)]
                    for kc in range(KC):
                        for n2 in range(2):
                            nc.tensor.matmul(
                                pk[n2][:],
                                xTo_sb[:, kc, b * XB:(b + 1) * XB],
                                wk_sb[:, kc, n2 * 512:(n2 + 1) * 512],
                                start=(kc == 0), stop=False)
                            nc.tensor.matmul(
                                pv[n2][:],
                                xTo_sb[:, kc, b * XB:(b + 1) * XB],
                                wv_sb[:, kc, n2 * 512:(n2 + 1) * 512],
                                start=(kc == 0), stop=False)
                    for n2 in range(2):
                        nc.tensor.matmul(
                            pk[n2][:], ones[:, :128],
                            bk_sb[:, n2 * 512:(n2 + 1) * 512],
                            start=False, stop=True)
                        nc.scalar.copy(
                            k_own[:, b, n2 * 512:(n2 + 1) * 512], pk[n2][:])
                        nc.tensor.matmul(
                            pv[n2][:], ones[:, :128],
                            bv_sb[:, n2 * 512:(n2 + 1) * 512],
                            start=False, stop=True)
                        nc.scalar.copy(
                            v_own[:, b, n2 * 512:(n2 + 1) * 512], pv[n2][:])
                        nc.vector.tensor_copy(
                            vq8[:, b, n2 * 512:(n2 + 1) * 512], pv[n2][:])

                if KPHASES >= 2 and not NOAG:
                    nc.gpsimd.dma_start(ag_in[:], vq8[:])
                    nc.gpsimd.collective_compute(
                        "AllGather", BYPASS,
                        replica_groups=[list(range(NCORES))],
                        ins=[ag_in.opt()], outs=[ag_out.opt()])
                elif KPHASES >= 2:
                    nc.gpsimd.dma_start(ag_out[0], vq8[:])

                # k^T v (own tokens), 2 heads at a time; diag blocks are M
                for b in range(B if KPHASES >= 2 else 0):
                    for hp4 in range(2):
                        psm = psM.tile([128, 512], F32, tag="psm")
                        for hq in range(4):
                            hp = hp4 * 4 + hq
                            nc.tensor.matmul(
                                psm[:, hq * 128:(hq + 1) * 128],
                                k_own[:, b, hp * 128:(hp + 1) * 128],
                                v_own[:, b, hp * 128:(hp + 1) * 128],
                                start=True, stop=True, skip_group_check=True)
                        # even heads: rows 0:64 cols 0:64 of each 128-block
                        src = psm[:].rearrange("p (q a d) -> p q a d", q=4, a=2)
                        dst = M_sb[:, hp4 * 4:(hp4 + 1) * 4, b, :]
                        nc.vector.tensor_copy(dst[0:64], src[0:64, :, 0, :])
                        nc.vector.tensor_copy(dst[64:128], src[64:128, :, 1, :])

                if KPHASES >= 2:
                    nc.vector.memset(M2blk[:], 0.0)
                if KPHASES >= 2 and not NOAR:
                    nc.gpsimd.dma_start(ar_in[:], M_sb[:])
                    nc.gpsimd.collective_compute(
                        "AllReduce", ADD,
                        replica_groups=[list(range(NCORES))],
                        ins=[ar_in.opt()], outs=[ar_out.opt()])
                    aro = ar_out[:].rearrange("p (hp b d) -> p hp b d", hp=HP, b=B)
                    nc.sync.dma_start(M2blk[0:64, :, :, 0:D], aro[0:64])
                    nc.sync.dma_start(M2blk[64:128, :, :, D:128], aro[64:128])
                elif KPHASES >= 2:
                    nc.vector.tensor_copy(M2blk[0:64, :, :, 0:D], M_sb[0:64])
                    nc.vector.tensor_copy(M2blk[64:128, :, :, D:128], M_sb[64:128])

            # ---------------- pe scores (fp8) ----------------
            # per x: 8 matmuls  scores[y128, bh] = pet_chunk^T @ q_x
            # pet partition-halves hold x and x+64 (so the two concurrent
            # row-group matmuls land in DIFFERENT psum tiles/banks); each
            # psum tile packs two consecutive x -> contiguous 128B evictions.
            with (
                tc.tile_pool(name="pepool", bufs=4) as pepool,
                tc.tile_pool(name="psS", bufs=2, space="PSUM") as psSp,
            ):
                for xg in range(XP // 2 if KPHASES >= 4 else 0):
                    p0 = 2 * xg
                    pt = pepool.tile([128, 2, S], F8, tag="pt")
                    for pp in range(2):
                        nc.sync.dma_start(pt[:, pp, :], pet[p0 + pp, :, :])
                    pssA = psSp.tile([128, YC, 2, H * B], F32, tag="pssA")
                    pssB = psSp.tile([128, YC, 2, H * B], F32, tag="pssB")
                    for yc in range(YC):
                        for pp in range(2):
                            for xpar in range(2):
                                sl = slice(xpar * 64, xpar * 64 + 64)
                                nc.tensor.matmul(
                                    (pssA if xpar == 0 else pssB)[:, yc, pp, :],
                                    pt[sl, pp, yc * 128:(yc + 1) * 128],
                                    qP8[sl, p0 + pp, :],
                                    start=True, stop=True,
                                    tile_position=(xpar * 64, 0),
                                    skip_group_check=True,
                                )
                    if xg % 3 == 2:   # spread evictions across DVE and ACT
                        nc.scalar.copy(S_sb[:, :, p0:p0 + 2, :], pssA[:])
                        nc.scalar.copy(S_sb[:, :, 64 + p0:64 + p0 + 2, :],
                                       pssB[:])
                    else:
                        nc.vector.tensor_copy(S_sb[:, :, p0:p0 + 2, :], pssA[:])
                        nc.vector.tensor_copy(S_sb[:, :, 64 + p0:64 + p0 + 2, :],
                                              pssB[:])

            # ---------------- attn = attn1 + attn2, out projection ----------
            with (
                tc.tile_pool(name="vpool", bufs=4) as vpool,
                tc.tile_pool(name="psA", bufs=2, space="PSUM") as psAp,
                tc.tile_pool(name="psO", bufs=2, space="PSUM") as psOp,
                tc.tile_pool(name="wopool", bufs=1) as wopool,
            ):
                wo_sb = wopool.tile([128, KC, E], BF, tag="wo")
                for kc in range(KC if KPHASES >= 6 else 0):
                    nc.sync.dma_start(wo_sb[:, kc, :], wo[kc * 128:(kc + 1) * 128, :])
                for b in range(B if KPHASES >= 5 else 0):
                    # psa: [ (h%2)*64+d, hp, x ] accumulated over yc + attn1
                    psa = psAp.tile([128, HP, XB], F32, tag="psa", name=f"psa{b}")
                    for yc in range(YC):
                        vsl = vpool.tile([128, E], F8, tag="vsl")
                        nc.gpsimd.dma_start(vsl[:], ag_out[yc, :, b, :])
                        for h in range(H):
                            par = h % 2
                            nc.tensor.matmul(
                                psa[par * 64:(par + 1) * 64, h // 2, :],
                                vsl[:, h * 64:(h + 1) * 64],
                                S_sb[:, yc, :, h * B + b],
                                start=(yc == 0), stop=False,
                                tile_position=(0, par * 64),
                                skip_group_check=True,
                            )
                    for hp in range(HP):  # attn1 = q @ (k^T v), block-diag M
                        nc.tensor.matmul(
                            psa[:, hp, :],
                            M2blk[:, hp, b, :],
                            qB[:, hp, b, :],
                            start=False, stop=True,
                            skip_group_check=True,
                        )
                    if KPHASES >= 6:
                        nc.vector.tensor_copy(attnT[:, :, b, :], psa[:])
                    if KPHASES >= 7:
                        pso = [psOp.tile([128, 512], F32, tag="pso",
                                         name=f"pso{b}_{i}") for i in range(2)]
                        for fc in range(KC):   # share attnT stationary
                            for n2 in range(2):
                                nc.tensor.matmul(
                                    pso[n2][:],
                                    attnT[:, fc, b, :],
                                    wo_sb[:, fc, n2 * 512:(n2 + 1) * 512],
                                    start=(fc == 0), stop=False,
                                )
                        for n2 in range(2):
                            nc.tensor.matmul(
                                pso[n2][:], ones[:, :128],
                                bo_sb[:, n2 * 512:(n2 + 1) * 512],
                                start=False, stop=True)
                            ost = stage.tile([128, 512], BF, tag="ost")
                            nc.scalar.copy(ost[:], pso[n2][:])
                            nc.sync.dma_start(
                                out[b * XB:(b + 1) * XB,
                                    n2 * 512:(n2 + 1) * 512],
                                ost[:])
    nc.compile()
    return nc


def shard_inputs(x, W_qkv, b_qkv, pe, W_out, b_out):
    bf = ml_dtypes.bfloat16
    f8 = ml_dtypes.float8_e4m3
    scale = D ** -0.5
    x2 = np.asarray(x, np.float32).reshape(B * S, E)
    xT = np.ascontiguousarray(x2.T).astype(bf)
    Wq = (np.asarray(W_qkv[:, :E], np.float32) * scale).astype(bf)
    Wk = np.asarray(W_qkv[:, E:2 * E], np.float32).astype(bf)
    Wv = np.asarray(W_qkv[:, 2 * E:], np.float32).astype(bf)
    Wo = np.asarray(W_out, np.float32).astype(bf)
    bqv = (np.asarray(b_qkv[:E], np.float32) * scale).astype(bf).reshape(1, E)
    bkv = np.asarray(b_qkv[E:2 * E], np.float32).astype(bf).reshape(1, E)
    bvv = np.asarray(b_qkv[2 * E:], np.float32).astype(bf).reshape(1, E)
    bov = np.asarray(b_out, np.float32).astype(bf).reshape(1, E)

    pe32 = np.asarray(pe, np.float32)
    in_maps = []
    for c in range(NCORES):
        x0 = c * XB
        # pet[p, xh*64+c, y] = pe[x0 + xh*64 + p, y, c]
        pet_c = np.ascontiguousarray(
            pe32[x0:x0 + XB].transpose(0, 2, 1).reshape(2, XP, D, S)
            .transpose(1, 0, 2, 3)).reshape(XP, 128, S)
        cols = (np.arange(B)[:, None] * S + (x0 + np.arange(XB))[None, :]).ravel()
        xTo = np.ascontiguousarray(xT[:, cols])
        in_maps.append({
            "xTo": xTo,
            "wq": Wq, "wk": Wk, "wv": Wv, "wo": Wo,
            "pet": pet_c.astype(f8),
            "bq": bqv, "bk": bkv, "bv": bvv, "bo": bov,
        })
    return in_maps


def kernel(x, W_qkv, b_qkv, pe, W_out, b_out, _trace=False):
    global _compiled
    if _compiled is None:
        _compiled = build_kernel()
    nc = _compiled
    in_maps = shard_inputs(x, W_qkv, b_qkv, pe, W_out, b_out)
    res = run_bass_kernel_spmd(nc, in_maps, core_ids=list(range(NCORES)),
                               trace=_trace)
    outs = res.results
    full = np.empty((B, S, E), np.float32)
    for c in range(NCORES):
        full[:, c * XB:(c + 1) * XB, :] = (
            outs[c]["out"].astype(np.float32).reshape(B, XB, E))
    if _trace:
        kernel.last_exec_time_ns = res.exec_time_ns
        kernel.last_profile = res.profile_json
    return full


# revision 37
# speedup vs baseline: 1.0928x; 1.0928x over previous
"""Trainium2 Bass kernel for nn_MultiHeadAttention_77412490543447 (v2).

reference:
  qkv = x @ W_qkv + b_qkv -> q,k,v  (B,H,S,D)
  scores = scale*(q k^T) + scale*einsum('xyc,bhxc->bhxy', pe, q)  [no softmax]
  out = (scores @ v) @ W_out + b_out

No softmax => (q k^T) @ v == q @ (k^T v): the qk path collapses to per-head
64x64 matrices (AllReduce'd across cores); only the pe term needs S*S work,
and it is ~2% of the output magnitude, so the whole pe path runs in fp8.

Sharding: query-position (x) blocks of 128 per core, 8 cores.  Each core
projects q/k/v for its OWN 512 tokens (N=512 matmuls), AllGathers v in fp8,
streams its pe slice (fp8) from DRAM, computes pe-scores with K=64
row-alternating matmuls (fp8 FWL weight loads), keeps scores fp8 in SBUF,
and accumulates attn = q@(k^T v) + scores^T_stationary @ v in PSUM [x, feat].
A DVE stream-transpose grid converts [x, feat] -> [feat, x] for the output
projection.
"""

import os
import numpy as np
import ml_dtypes

import concourse.bass as bass
import concourse.bacc as bacc
import concourse.mybir as mybir
import concourse.tile as tile
from concourse.bass_utils import run_bass_kernel_spmd

BF = mybir.dt.bfloat16
F8 = mybir.dt.float8e4
F32 = mybir.dt.float32
ADD = mybir.AluOpType.add
BYPASS = mybir.AluOpType.bypass

B, S, E = 4, 1024, 1024
H, D = 16, 64
HP = H // 2               # head pairs
NCORES = 8
XB = S // NCORES          # 128 query positions per core
OWN = B * XB              # 512 own tokens
KC = E // 128             # 8 contraction chunks
YC = S // 128             # 8 key-position chunks
XP = XB // 2              # 64 x-pairs

_compiled = None
KPHASES = int(os.environ.get('KPHASES', '9'))
NOAG = int(os.environ.get('NOAG', '0'))
NOAR = int(os.environ.get('NOAR', '0'))


def build_kernel():
    nc = bacc.Bacc(None, target_bir_lowering=False)

    xTo = nc.dram_tensor("xTo", [E, OWN], BF, kind="ExternalInput")
    wq = nc.dram_tensor("wq", [E, E], BF, kind="ExternalInput")
    wk = nc.dram_tensor("wk", [E, E], BF, kind="ExternalInput")
    wv = nc.dram_tensor("wv", [E, E], BF, kind="ExternalInput")
    wo = nc.dram_tensor("wo", [E, E], BF, kind="ExternalInput")
    pet = nc.dram_tensor("pet", [XP, 128, S], F8, kind="ExternalInput")
    bq = nc.dram_tensor("bq", [1, E], BF, kind="ExternalInput")
    bk = nc.dram_tensor("bk", [1, E], BF, kind="ExternalInput")
    bv = nc.dram_tensor("bv", [1, E], BF, kind="ExternalInput")
    bo = nc.dram_tensor("bo", [1, E], BF, kind="ExternalInput")
    out = nc.dram_tensor("out", [OWN, E], BF, kind="ExternalOutput")

    with tile.TileContext(nc) as tc:
        with (
            tc.tile_pool(name="dram", bufs=1, space="DRAM") as dram,
            tc.tile_pool(name="const", bufs=1) as const,
            tc.tile_pool(name="res", bufs=1) as res,
            tc.tile_pool(name="stage", bufs=2) as stage,
        ):
            # collective bounce buffers (DRAM)
            ag_in = dram.tile([128, B, E], F8)            # own v shard (fp8)
            ag_out = dram.tile([NCORES, 128, B, E], F8, addr_space="Shared")
            ar_in = dram.tile([128, HP * B * D], BF)      # local k^T v
            ar_out = dram.tile([128, HP * B * D], BF, addr_space="Shared")

            ones = const.tile([1, 512], BF)
            nc.vector.memset(ones[:], 1.0)
            bq_sb = const.tile([1, E], BF, tag="bq")
            bk_sb = const.tile([1, E], BF, tag="bk")
            bv_sb = const.tile([1, E], BF, tag="bv")
            bo_sb = const.tile([1, E], BF, tag="bo")
            nc.gpsimd.dma_start(bq_sb[:], bq[:])
            nc.gpsimd.dma_start(bk_sb[:], bk[:])
            nc.gpsimd.dma_start(bv_sb[:], bv[:])
            nc.gpsimd.dma_start(bo_sb[:], bo[:])

            # resident tensors
            k_own = res.tile([128, B, E], BF, tag="k_own")
            v_own = res.tile([128, B, E], BF, tag="v_own")
            vq8 = res.tile([128, B, E], F8, tag="vq8")
            # qB: [ (h%2)*64+c, hp, b, x ] bf16  (attn1 lhsT)
            qB = res.tile([128, HP, B, XB], BF, tag="qB")
            # qP8: [ (x//64)*64+c, x%64, bh ] fp8  (pe-score rhs); bh = h*B+b
            qP8 = res.tile([128, XP, H * B], F8, tag="qP8")
            # M (k^T v): [ (h%2)*64+c, hp, b, d ]
            M_sb = res.tile([128, HP, B, D], BF, tag="M_sb")
            # block-diagonal M for K=128 attn1: [ c-pair, hp, b, d-pair ]
            M2blk = res.tile([128, HP, B, 128], BF, tag="M2blk")
            # scores: [ y%128, yc, x, bh ] fp8 (bh innermost: contiguous evict)
            S_sb = res.tile([128, YC, XB, H * B], F8, tag="S_sb")
            # attnT: [ f%128 = (h%2)*64+d, f//128 = hp, b, x ] bf16
            attnT = res.tile([128, KC, B, XB], BF, tag="attnT")

            xTo_sb = res.tile([128, KC, OWN], BF, tag="xTo")
            for kc in range(KC):
                nc.gpsimd.dma_start(xTo_sb[:, kc, :], xTo[kc * 128:(kc + 1) * 128, :])

            # ---------------- projections + k^T v ----------------
            with (
                tc.tile_pool(name="wpool", bufs=3) as wpool,
                tc.tile_pool(name="psP", bufs=6, space="PSUM") as psP,
                tc.tile_pool(name="psM", bufs=2, space="PSUM") as psM,
            ):
                wk_sb = wpool.tile([128, KC, E], BF, tag="w", name="wk_sb")
                wv_sb = wpool.tile([128, KC, E], BF, tag="w", name="wv_sb")
                wq_sb = wpool.tile([128, KC, E], BF, tag="w", name="wq_sb")
                for kc in range(KC):
                    nc.sync.dma_start(wq_sb[:, kc, :], wq[kc * 128:(kc + 1) * 128, :])
                for kc in range(KC):
                    nc.sync.dma_start(wk_sb[:, kc, :], wk[kc * 128:(kc + 1) * 128, :])
                for kc in range(KC):
                    nc.sync.dma_start(wv_sb[:, kc, :], wv[kc * 128:(kc + 1) * 128, :])
                # q projection (feature-major: [feat, tok]), scale pre-folded
                for hp in range(HP if KPHASES >= 3 else 0):
                    ps = psP.tile([128, 512], F32, tag="ps")
                    for kc in range(KC):
                        nc.tensor.matmul(
                            ps[:],
                            wq_sb[:, kc, hp * 128:(hp + 1) * 128],
                            xTo_sb[:, kc, :],
                            start=(kc == 0), stop=False,
                        )
                    nc.tensor.matmul(
                        ps[:], bq_sb[:, hp * 128:(hp + 1) * 128], ones[:],
                        start=False, stop=True)
                    # qB: [(h%2)*64+c, hp, b, x] <- ps [(h%2)*64+c, (b x)]
                    qb_eng = nc.scalar if hp % 2 == 0 else nc.vector
                    qb_copy = (nc.scalar.copy if hp % 2 == 0
                               else nc.vector.tensor_copy)
                    qb_copy(
                        qB[:, hp, :, :],
                        ps[:].rearrange("p (b x) -> p b x", b=B))
                    # qP8: [(x//64)*64+c, x%64, h*B+b] <- ps[par*64+c, (b, x)]
                    for par in range(2):
                        h = 2 * hp + par
                        for xh in range(2):
                            src = ps[par * 64:(par + 1) * 64, :].rearrange(
                                "c (b xh p) -> c xh b p", b=B, xh=2)[
                                :, xh, :, :]
                            dst = qP8[xh * 64:(xh + 1) * 64, :, :].rearrange(
                                "c p (h b) -> c h b p", h=H)[:, h, :, :]
                            if hp % 2 == 0:
                                nc.vector.tensor_copy(dst, src)
                            else:
                                nc.scalar.copy(dst, src)

                # k+v projections fused: share the xTo stationary across the
                # 4 output halves per contraction chunk
                for b in range(B if KPHASES >= 1 else 0):
                    pk = [psP.tile([128, 512], F32, tag="ps", name=f"pk{b}_{i}")
                          for i in range(2)]
                    pv = [psP.tile([128, 512], F32, tag="ps", name=f"pv{b}_{i}")
                          for i in range(2)]
                    for kc in range(KC):
                        for n2 in range(2):
                            nc.tensor.matmul(
                                pk[n2][:],
                                xTo_sb[:, kc, b * XB:(b + 1) * XB],
                                wk_sb[:, kc, n2 * 512:(n2 + 1) * 512],
                                start=(kc == 0), stop=False)
                            nc.tensor.matmul(
                                pv[n2][:],
                                xTo_sb[:, kc, b * XB:(b + 1) * XB],
                                wv_sb[:, kc, n2 * 512:(n2 + 1) * 512],
                                start=(kc == 0), stop=False)
                    for n2 in range(2):
                        nc.tensor.matmul(
                            pk[n2][:], ones[:, :128],
                            bk_sb[:, n2 * 512:(n2 + 1) * 512],
                            start=False, stop=True)
                        nc.scalar.copy(
                            k_own[:, b, n2 * 512:(n2 + 1) * 512], pk[n2][:])
                        nc.tensor.matmul(
                            pv[n2][:], ones[:, :128],
                            bv_sb[:, n2 * 512:(n2 + 1) * 512],
                            start=False, stop=True)
                        nc.scalar.copy(
                            v_own[:, b, n2 * 512:(n2 + 1) * 512], pv[n2][:])
                        nc.vector.tensor_copy(
                            vq8[:, b, n2 * 512:(n2 + 1) * 512], pv[n2][:])

                if KPHASES >= 2 and not NOAG:
                    nc.gpsimd.dma_start(ag_in[:], vq8[:])
                    nc.gpsimd.collective_compute(
                        "AllGather", BYPASS,
                        replica_groups=[list(range(NCORES))],
                        ins=[ag_in.opt()], outs=[ag_out.opt()])
                elif KPHASES >= 2:
                    nc.gpsimd.dma_start(ag_out[0], vq8[:])

                # k^T v (own tokens), 2 heads at a time; diag blocks are M
                for b in range(B if KPHASES >= 2 else 0):
                    for hp4 in range(2):
                        psm = psM.tile([128, 512], F32, tag="psm")
                        for hq in range(4):
                            hp = hp4 * 4 + hq
                            nc.tensor.matmul(
                                psm[:, hq * 128:(hq + 1) * 128],
                                k_own[:, b, hp * 128:(hp + 1) * 128],
                                v_own[:, b, hp * 128:(hp + 1) * 128],
                                start=True, stop=True, skip_group_check=True)
                        # even heads: rows 0:64 cols 0:64 of each 128-block
                        src = psm[:].rearrange("p (q a d) -> p q a d", q=4, a=2)
                        dst = M_sb[:, hp4 * 4:(hp4 + 1) * 4, b, :]
                        nc.vector.tensor_copy(dst[0:64], src[0:64, :, 0, :])
                        nc.vector.tensor_copy(dst[64:128], src[64:128, :, 1, :])

                if KPHASES >= 2:
                    nc.vector.memset(M2blk[:], 0.0)
                if KPHASES >= 2 and not NOAR:
                    nc.gpsimd.dma_start(ar_in[:], M_sb[:])
                    nc.gpsimd.collective_compute(
                        "AllReduce", ADD,
                        replica_groups=[list(range(NCORES))],
                        ins=[ar_in.opt()], outs=[ar_out.opt()])
                    aro = ar_out[:].rearrange("p (hp b d) -> p hp b d", hp=HP, b=B)
                    nc.sync.dma_start(M2blk[0:64, :, :, 0:D], aro[0:64])
                    nc.sync.dma_start(M2blk[64:128, :, :, D:128], aro[64:128])
                elif KPHASES >= 2:
                    nc.vector.tensor_copy(M2blk[0:64, :, :, 0:D], M_sb[0:64])
                    nc.vector.tensor_copy(M2blk[64:128, :, :, D:128], M_sb[64:128])

            # ---------------- pe scores (fp8) ----------------
            # per x: 8 matmuls  scores[y128, bh] = pet_chunk^T @ q_x
            # pet partition-halves hold x and x+64 (so the two concurrent
            # row-group matmuls land in DIFFERENT psum tiles/banks); each
            # psum tile packs two consecutive x -> contiguous 128B evictions.
            with (
                tc.tile_pool(name="pepool", bufs=4) as pepool,
                tc.tile_pool(name="psS", bufs=2, space="PSUM") as psSp,
            ):
                for xg in range(XP // 2 if KPHASES >= 4 else 0):
                    p0 = 2 * xg
                    pt = pepool.tile([128, 2, S], F8, tag="pt")
                    for pp in range(2):
                        nc.sync.dma_start(pt[:, pp, :], pet[p0 + pp, :, :])
                    pssA = psSp.tile([128, YC, 2, H * B], F32, tag="pssA")
                    pssB = psSp.tile([128, YC, 2, H * B], F32, tag="pssB")
                    for yc in range(YC):
                        for pp in range(2):
                            for xpar in range(2):
                                sl = slice(xpar * 64, xpar * 64 + 64)
                                nc.tensor.matmul(
                                    (pssA if xpar == 0 else pssB)[:, yc, pp, :],
                                    pt[sl, pp, yc * 128:(yc + 1) * 128],
                                    qP8[sl, p0 + pp, :],
                                    start=True, stop=True,
                                    tile_position=(xpar * 64, 0),
                                    skip_group_check=True,
                                )
                    if xg % 3 == 2:   # spread evictions across DVE and ACT
                        nc.scalar.copy(S_sb[:, :, p0:p0 + 2, :], pssA[:])
                        nc.scalar.copy(S_sb[:, :, 64 + p0:64 + p0 + 2, :],
                                       pssB[:])
                    else:
                        nc.vector.tensor_copy(S_sb[:, :, p0:p0 + 2, :], pssA[:])
                        nc.vector.tensor_copy(S_sb[:, :, 64 + p0:64 + p0 + 2, :],
                                              pssB[:])

            # ---------------- attn = attn1 + attn2, out projection ----------
            with (
                tc.tile_pool(name="vpool", bufs=4) as vpool,
                tc.tile_pool(name="psA", bufs=2, space="PSUM") as psAp,
                tc.tile_pool(name="psO", bufs=2, space="PSUM") as psOp,
                tc.tile_pool(name="wopool", bufs=1) as wopool,
            ):
                wo_sb = wopool.tile([128, KC, E], BF, tag="wo")
                for kc in range(KC if KPHASES >= 6 else 0):
                    nc.sync.dma_start(wo_sb[:, kc, :], wo[kc * 128:(kc + 1) * 128, :])
                for b in range(B if KPHASES >= 5 else 0):
                    # psa: [ (h%2)*64+d, hp, x ] accumulated over yc + attn1
                    psa = psAp.tile([128, HP, XB], F32, tag="psa", name=f"psa{b}")
                    for yc in range(YC):
                        vsl = vpool.tile([128, E], F8, tag="vsl")
                        nc.gpsimd.dma_start(vsl[:], ag_out[yc, :, b, :])
                        for h in range(H):
                            par = h % 2
                            nc.tensor.matmul(
                                psa[par * 64:(par + 1) * 64, h // 2, :],
                                vsl[:, h * 64:(h + 1) * 64],
                                S_sb[:, yc, :, h * B + b],
                                start=(yc == 0), stop=False,
                                tile_position=(0, par * 64),
                                skip_group_check=True,
                            )
                    for hp in range(HP):  # attn1 = q @ (k^T v), block-diag M
                        nc.tensor.matmul(
                            psa[:, hp, :],
                            M2blk[:, hp, b, :],
                            qB[:, hp, b, :],
                            start=False, stop=True,
                            skip_group_check=True,
                        )
                    if KPHASES >= 6:
                        nc.vector.tensor_copy(attnT[:, :, b, :], psa[:])
                    if KPHASES >= 7:
                        pso = [psOp.tile([128, 512], F32, tag="pso",
                                         name=f"pso{b}_{i}") for i in range(2)]
                        for fc in range(KC):   # share attnT stationary
                            for n2 in range(2):
                                nc.tensor.matmul(
                                    pso[n2][:],
                                    attnT[:, fc, b, :],
                                    wo_sb[:, fc, n2 * 512:(n2 + 1) * 512],
                                    start=(fc == 0), stop=False,
                                )
                        for n2 in range(2):
                            nc.tensor.matmul(
                                pso[n2][:], ones[:, :128],
                                bo_sb[:, n2 * 512:(n2 + 1) * 512],
                                start=False, stop=True)
                            ost = stage.tile([128, 512], BF, tag="ost")
                            nc.scalar.copy(ost[:], pso[n2][:])
                            nc.sync.dma_start(
                                out[b * XB:(b + 1) * XB,
                                    n2 * 512:(n2 + 1) * 512],
                                ost[:])
    nc.compile()
    return nc


def shard_inputs(x, W_qkv, b_qkv, pe, W_out, b_out):
    bf = ml_dtypes.bfloat16
    f8 = ml_dtypes.float8_e4m3
    scale = D ** -0.5
    x2 = np.asarray(x, np.float32).reshape(B * S, E)
    xT = np.ascontiguousarray(x2.T).astype(bf)
    Wq = (np.asarray(W_qkv[:, :E], np.float32) * scale).astype(bf)
    Wk = np.asarray(W_qkv[:, E:2 * E], np.float32).astype(bf)
    Wv = np.asarray(W_qkv[:, 2 * E:], np.float32).astype(bf)
    Wo = np.asarray(W_out, np.float32).astype(bf)
    bqv = (np.asarray(b_qkv[:E], np.float32) * scale).astype(bf).reshape(1, E)
    bkv = np.asarray(b_qkv[E:2 * E], np.float32).astype(bf).reshape(1, E)
    bvv = np.asarray(b_qkv[2 * E:], np.float32).astype(bf).reshape(1, E)
    bov = np.asarray(b_out, np.float32).astype(bf).reshape(1, E)

    pe32 = np.asarray(pe, np.float32)
    in_maps = []
    for c in range(NCORES):
        x0 = c * XB
        # pet[p, xh*64+c, y] = pe[x0 + xh*64 + p, y, c]
        pet_c = np.ascontiguousarray(
            pe32[x0:x0 + XB].transpose(0, 2, 1).reshape(2, XP, D, S)
            .transpose(1, 0, 2, 3)).reshape(XP, 128, S)
        cols = (np.arange(B)[:, None] * S + (x0 + np.arange(XB))[None, :]).ravel()
        xTo = np.ascontiguousarray(xT[:, cols])
        in_maps.append({
            "xTo": xTo,
            "wq": Wq, "wk": Wk, "wv": Wv, "wo": Wo,
            "pet": pet_c.astype(f8),
            "bq": bqv, "bk": bkv, "bv": bvv, "bo": bov,
        })
    return in_maps


def kernel(x, W_qkv, b_qkv, pe, W_out, b_out, _trace=False):
    global _compiled
    if _compiled is None:
        _compiled = build_kernel()
    nc = _compiled
    in_maps = shard_inputs(x, W_qkv, b_qkv, pe, W_out, b_out)
    res = run_bass_kernel_spmd(nc, in_maps, core_ids=list(range(NCORES)),
                               trace=_trace)
    outs = res.results
    full = np.empty((B, S, E), np.float32)
    for c in range(NCORES):
        full[:, c * XB:(c + 1) * XB, :] = (
            outs[c]["out"].astype(np.float32).reshape(B, XB, E))
    if _trace:
        kernel.last_exec_time_ns = res.exec_time_ns
        kernel.last_profile = res.profile_json
    return full


# revision 38
# speedup vs baseline: 1.1580x; 1.0596x over previous
"""Trainium2 Bass kernel for nn_MultiHeadAttention_77412490543447 (v2).

reference:
  qkv = x @ W_qkv + b_qkv -> q,k,v  (B,H,S,D)
  scores = scale*(q k^T) + scale*einsum('xyc,bhxc->bhxy', pe, q)  [no softmax]
  out = (scores @ v) @ W_out + b_out

No softmax => (q k^T) @ v == q @ (k^T v): the qk path collapses to per-head
64x64 matrices (AllReduce'd across cores); only the pe term needs S*S work,
and it is ~2% of the output magnitude, so the whole pe path runs in fp8.

Sharding: query-position (x) blocks of 128 per core, 8 cores.  Each core
projects q/k/v for its OWN 512 tokens (N=512 matmuls), AllGathers v in fp8,
streams its pe slice (fp8) from DRAM, computes pe-scores with K=64
row-alternating matmuls (fp8 FWL weight loads), keeps scores fp8 in SBUF,
and accumulates attn = q@(k^T v) + scores^T_stationary @ v in PSUM [x, feat].
A DVE stream-transpose grid converts [x, feat] -> [feat, x] for the output
projection.
"""

import os
import numpy as np
import ml_dtypes

import concourse.bass as bass
import concourse.bacc as bacc
import concourse.mybir as mybir
import concourse.tile as tile
from concourse.bass_utils import run_bass_kernel_spmd

BF = mybir.dt.bfloat16
F8 = mybir.dt.float8e4
F32 = mybir.dt.float32
ADD = mybir.AluOpType.add
BYPASS = mybir.AluOpType.bypass

B, S, E = 4, 1024, 1024
H, D = 16, 64
HP = H // 2               # head pairs
NCORES = 8
XB = S // NCORES          # 128 query positions per core
OWN = B * XB              # 512 own tokens
KC = E // 128             # 8 contraction chunks
YC = S // 128             # 8 key-position chunks
XP = XB // 2              # 64 x-pairs

_compiled = None
KPHASES = int(os.environ.get('KPHASES', '9'))
NOAG = int(os.environ.get('NOAG', '0'))
NOAR = int(os.environ.get('NOAR', '0'))


def build_kernel():
    nc = bacc.Bacc(None, target_bir_lowering=False)

    xTo = nc.dram_tensor("xTo", [E, OWN], BF, kind="ExternalInput")
    wq = nc.dram_tensor("wq", [E, E], BF, kind="ExternalInput")
    wk = nc.dram_tensor("wk", [E, E], BF, kind="ExternalInput")
    wv = nc.dram_tensor("wv", [E, E], BF, kind="ExternalInput")
    wo = nc.dram_tensor("wo", [E, E], BF, kind="ExternalInput")
    pet = nc.dram_tensor("pet", [XP, 128, S], F8, kind="ExternalInput")
    bq = nc.dram_tensor("bq", [1, E], BF, kind="ExternalInput")
    bk = nc.dram_tensor("bk", [1, E], BF, kind="ExternalInput")
    bv = nc.dram_tensor("bv", [1, E], BF, kind="ExternalInput")
    bo = nc.dram_tensor("bo", [1, E], BF, kind="ExternalInput")
    out = nc.dram_tensor("out", [OWN, E], BF, kind="ExternalOutput")

    with tile.TileContext(nc) as tc:
        with (
            tc.tile_pool(name="dram", bufs=1, space="DRAM") as dram,
            tc.tile_pool(name="const", bufs=1) as const,
            tc.tile_pool(name="res", bufs=1) as res,
            tc.tile_pool(name="stage", bufs=2) as stage,
        ):
            # collective bounce buffers (DRAM)
            ag_in = dram.tile([128, B, E], F8)            # own v shard (fp8)
            ag_out = dram.tile([NCORES, 128, B, E], F8, addr_space="Shared")
            ar_in = dram.tile([128, HP * B * D], BF)      # local k^T v
            ar_out = dram.tile([128, HP * B * D], BF, addr_space="Shared")

            ones = const.tile([1, 512], BF)
            nc.vector.memset(ones[:], 1.0)
            bq_sb = const.tile([1, E], BF, tag="bq")
            bk_sb = const.tile([1, E], BF, tag="bk")
            bv_sb = const.tile([1, E], BF, tag="bv")
            bo_sb = const.tile([1, E], BF, tag="bo")
            nc.gpsimd.dma_start(bq_sb[:], bq[:])
            nc.gpsimd.dma_start(bk_sb[:], bk[:])
            nc.gpsimd.dma_start(bv_sb[:], bv[:])
            nc.gpsimd.dma_start(bo_sb[:], bo[:])

            # resident tensors
            k_own = res.tile([128, B, E], BF, tag="k_own")
            v_own = res.tile([128, B, E], BF, tag="v_own")
            vq8 = res.tile([128, B, E], F8, tag="vq8")
            # qB: [ (h%2)*64+c, hp, b, x ] bf16  (attn1 lhsT)
            qB = res.tile([128, HP, B, XB], BF, tag="qB")
            # qP8: [ (x//64)*64+c, x%64, bh ] fp8  (pe-score rhs); bh = h*B+b
            qP8 = res.tile([128, XP, H * B], F8, tag="qP8")
            # M (k^T v): [ (h%2)*64+c, hp, b, d ]
            M_sb = res.tile([128, HP, B, D], BF, tag="M_sb")
            # block-diagonal M for K=128 attn1: [ c-pair, hp, b, d-pair ]
            M2blk = res.tile([128, HP, B, 128], BF, tag="M2blk")
            # scores: [ y%128, yc, x, bh ] fp8 (bh innermost: contiguous evict)
            S_sb = res.tile([128, YC, XB, H * B], F8, tag="S_sb")
            # attnT: [ f%128 = (h%2)*64+d, f//128 = hp, b, x ] bf16
            attnT = res.tile([128, KC, B, XB], BF, tag="attnT")

            xTo_sb = res.tile([128, KC, OWN], BF, tag="xTo")
            for kc in range(KC):
                nc.gpsimd.dma_start(xTo_sb[:, kc, :], xTo[kc * 128:(kc + 1) * 128, :])

            # ---------------- projections + k^T v ----------------
            with (
                tc.tile_pool(name="wpool", bufs=3) as wpool,
                tc.tile_pool(name="psP", bufs=6, space="PSUM") as psP,
                tc.tile_pool(name="psM", bufs=2, space="PSUM") as psM,
            ):
                wk_sb = wpool.tile([128, KC, E], BF, tag="w", name="wk_sb")
                wv_sb = wpool.tile([128, KC, E], BF, tag="w", name="wv_sb")
                wq_sb = wpool.tile([128, KC, E], BF, tag="w", name="wq_sb")
                for kc in range(KC):
                    nc.sync.dma_start(wk_sb[:, kc, :], wk[kc * 128:(kc + 1) * 128, :])
                for kc in range(KC):
                    nc.sync.dma_start(wv_sb[:, kc, :], wv[kc * 128:(kc + 1) * 128, :])
                for kc in range(KC):
                    nc.sync.dma_start(wq_sb[:, kc, :], wq[kc * 128:(kc + 1) * 128, :])
                # k+v projections fused: share the xTo stationary across the
                # 4 output halves per contraction chunk
                for b in range(B if KPHASES >= 1 else 0):
                    pk = [psP.tile([128, 512], F32, tag="ps", name=f"pk{b}_{i}")
                          for i in range(2)]
                    pv = [psP.tile([128, 512], F32, tag="ps", name=f"pv{b}_{i}")
                          for i in range(2)]
                    for kc in range(KC):
                        for n2 in range(2):
                            nc.tensor.matmul(
                                pk[n2][:],
                                xTo_sb[:, kc, b * XB:(b + 1) * XB],
                                wk_sb[:, kc, n2 * 512:(n2 + 1) * 512],
                                start=(kc == 0), stop=False)
                            nc.tensor.matmul(
                                pv[n2][:],
                                xTo_sb[:, kc, b * XB:(b + 1) * XB],
                                wv_sb[:, kc, n2 * 512:(n2 + 1) * 512],
                                start=(kc == 0), stop=False)
                    for n2 in range(2):
                        nc.tensor.matmul(
                            pk[n2][:], ones[:, :128],
                            bk_sb[:, n2 * 512:(n2 + 1) * 512],
                            start=False, stop=True)
                        nc.scalar.copy(
                            k_own[:, b, n2 * 512:(n2 + 1) * 512], pk[n2][:])
                        nc.tensor.matmul(
                            pv[n2][:], ones[:, :128],
                            bv_sb[:, n2 * 512:(n2 + 1) * 512],
                            start=False, stop=True)
                        nc.scalar.copy(
                            v_own[:, b, n2 * 512:(n2 + 1) * 512], pv[n2][:])
                        nc.vector.tensor_copy(
                            vq8[:, b, n2 * 512:(n2 + 1) * 512], pv[n2][:])

                # q projection (feature-major: [feat, tok]), scale pre-folded
                for hp in range(HP if KPHASES >= 3 else 0):
                    ps = psP.tile([128, 512], F32, tag="ps")
                    for kc in range(KC):
                        nc.tensor.matmul(
                            ps[:],
                            wq_sb[:, kc, hp * 128:(hp + 1) * 128],
                            xTo_sb[:, kc, :],
                            start=(kc == 0), stop=False,
                        )
                    nc.tensor.matmul(
                        ps[:], bq_sb[:, hp * 128:(hp + 1) * 128], ones[:],
                        start=False, stop=True)
                    # qB: [(h%2)*64+c, hp, b, x] <- ps [(h%2)*64+c, (b x)]
                    qb_eng = nc.scalar if hp % 2 == 0 else nc.vector
                    qb_copy = (nc.scalar.copy if hp % 2 == 0
                               else nc.vector.tensor_copy)
                    qb_copy(
                        qB[:, hp, :, :],
                        ps[:].rearrange("p (b x) -> p b x", b=B))
                    # qP8: [(x//64)*64+c, x%64, h*B+b] <- ps[par*64+c, (b, x)]
                    for par in range(2):
                        h = 2 * hp + par
                        for xh in range(2):
                            src = ps[par * 64:(par + 1) * 64, :].rearrange(
                                "c (b xh p) -> c xh b p", b=B, xh=2)[
                                :, xh, :, :]
                            dst = qP8[xh * 64:(xh + 1) * 64, :, :].rearrange(
                                "c p (h b) -> c h b p", h=H)[:, h, :, :]
                            if hp % 2 == 0:
                                nc.vector.tensor_copy(dst, src)
                            else:
                                nc.scalar.copy(dst, src)

                if KPHASES >= 2 and not NOAG:
                    nc.gpsimd.dma_start(ag_in[:], vq8[:])
                    nc.gpsimd.collective_compute(
                        "AllGather", BYPASS,
                        replica_groups=[list(range(NCORES))],
                        ins=[ag_in.opt()], outs=[ag_out.opt()])
                elif KPHASES >= 2:
                    nc.gpsimd.dma_start(ag_out[0], vq8[:])

                # k^T v (own tokens), 2 heads at a time; diag blocks are M
                for b in range(B if KPHASES >= 2 else 0):
                    for hp4 in range(2):
                        psm = psM.tile([128, 512], F32, tag="psm")
                        for hq in range(4):
                            hp = hp4 * 4 + hq
                            nc.tensor.matmul(
                                psm[:, hq * 128:(hq + 1) * 128],
                                k_own[:, b, hp * 128:(hp + 1) * 128],
                                v_own[:, b, hp * 128:(hp + 1) * 128],
                                start=True, stop=True, skip_group_check=True)
                        # even heads: rows 0:64 cols 0:64 of each 128-block
                        src = psm[:].rearrange("p (q a d) -> p q a d", q=4, a=2)
                        dst = M_sb[:, hp4 * 4:(hp4 + 1) * 4, b, :]
                        nc.vector.tensor_copy(dst[0:64], src[0:64, :, 0, :])
                        nc.vector.tensor_copy(dst[64:128], src[64:128, :, 1, :])

                if KPHASES >= 2:
                    nc.vector.memset(M2blk[:], 0.0)
                if KPHASES >= 2 and not NOAR:
                    nc.gpsimd.dma_start(ar_in[:], M_sb[:])
                    nc.gpsimd.collective_compute(
                        "AllReduce", ADD,
                        replica_groups=[list(range(NCORES))],
                        ins=[ar_in.opt()], outs=[ar_out.opt()])
                    aro = ar_out[:].rearrange("p (hp b d) -> p hp b d", hp=HP, b=B)
                    nc.sync.dma_start(M2blk[0:64, :, :, 0:D], aro[0:64])
                    nc.sync.dma_start(M2blk[64:128, :, :, D:128], aro[64:128])
                elif KPHASES >= 2:
                    nc.vector.tensor_copy(M2blk[0:64, :, :, 0:D], M_sb[0:64])
                    nc.vector.tensor_copy(M2blk[64:128, :, :, D:128], M_sb[64:128])

            # ---------------- pe scores (fp8) ----------------
            # per x: 8 matmuls  scores[y128, bh] = pet_chunk^T @ q_x
            # pet partition-halves hold x and x+64 (so the two concurrent
            # row-group matmuls land in DIFFERENT psum tiles/banks); each
            # psum tile packs two consecutive x -> contiguous 128B evictions.
            with (
                tc.tile_pool(name="pepool", bufs=4) as pepool,
                tc.tile_pool(name="psS", bufs=2, space="PSUM") as psSp,
            ):
                for xg in range(XP // 2 if KPHASES >= 4 else 0):
                    p0 = 2 * xg
                    pt = pepool.tile([128, 2, S], F8, tag="pt")
                    for pp in range(2):
                        nc.sync.dma_start(pt[:, pp, :], pet[p0 + pp, :, :])
                    pssA = psSp.tile([128, YC, 2, H * B], F32, tag="pssA")
                    pssB = psSp.tile([128, YC, 2, H * B], F32, tag="pssB")
                    for yc in range(YC):
                        for pp in range(2):
                            for xpar in range(2):
                                sl = slice(xpar * 64, xpar * 64 + 64)
                                nc.tensor.matmul(
                                    (pssA if xpar == 0 else pssB)[:, yc, pp, :],
                                    pt[sl, pp, yc * 128:(yc + 1) * 128],
                                    qP8[sl, p0 + pp, :],
                                    start=True, stop=True,
                                    tile_position=(xpar * 64, 0),
                                    skip_group_check=True,
                                )
                    if xg % 3 == 2:   # spread evictions across DVE and ACT
                        nc.scalar.copy(S_sb[:, :, p0:p0 + 2, :], pssA[:])
                        nc.scalar.copy(S_sb[:, :, 64 + p0:64 + p0 + 2, :],
                                       pssB[:])
                    else:
                        nc.vector.tensor_copy(S_sb[:, :, p0:p0 + 2, :], pssA[:])
                        nc.vector.tensor_copy(S_sb[:, :, 64 + p0:64 + p0 + 2, :],
                                              pssB[:])

            # ---------------- attn = attn1 + attn2, out projection ----------
            with (
                tc.tile_pool(name="vpool", bufs=4) as vpool,
                tc.tile_pool(name="psA", bufs=2, space="PSUM") as psAp,
                tc.tile_pool(name="psO", bufs=2, space="PSUM") as psOp,
                tc.tile_pool(name="wopool", bufs=1) as wopool,
            ):
                wo_sb = wopool.tile([128, KC, E], BF, tag="wo")
                for kc in range(KC if KPHASES >= 6 else 0):
                    nc.sync.dma_start(wo_sb[:, kc, :], wo[kc * 128:(kc + 1) * 128, :])
                for b in range(B if KPHASES >= 5 else 0):
                    # psa: [ (h%2)*64+d, hp, x ] accumulated over yc + attn1
                    psa = psAp.tile([128, HP, XB], F32, tag="psa", name=f"psa{b}")
                    for yc in range(YC):
                        vsl = vpool.tile([128, E], F8, tag="vsl")
                        nc.gpsimd.dma_start(vsl[:], ag_out[yc, :, b, :])
                        for h in range(H):
                            par = h % 2
                            nc.tensor.matmul(
                                psa[par * 64:(par + 1) * 64, h // 2, :],
                                vsl[:, h * 64:(h + 1) * 64],
                                S_sb[:, yc, :, h * B + b],
                                start=(yc == 0), stop=False,
                                tile_position=(0, par * 64),
                                skip_group_check=True,
                            )
                    for hp in range(HP):  # attn1 = q @ (k^T v), block-diag M
                        nc.tensor.matmul(
                            psa[:, hp, :],
                            M2blk[:, hp, b, :],
                            qB[:, hp, b, :],
                            start=False, stop=True,
                            skip_group_check=True,
                        )
                    if KPHASES >= 6:
                        nc.vector.tensor_copy(attnT[:, :, b, :], psa[:])
                    if KPHASES >= 7:
                        pso = [psOp.tile([128, 512], F32, tag="pso",
                                         name=f"pso{b}_{i}") for i in range(2)]
                        for fc in range(KC):   # share attnT stationary
                            for n2 in range(2):
                                nc.tensor.matmul(
                                    pso[n2][:],
                                    attnT[:, fc, b, :],
                                    wo_sb[:, fc, n2 * 512:(n2 + 1) * 512],
                                    start=(fc == 0), stop=False,
                                )
                        for n2 in range(2):
                            nc.tensor.matmul(
                                pso[n2][:], ones[:, :128],
                                bo_sb[:, n2 * 512:(n2 + 1) * 512],
                                start=False, stop=True)
                            ost = stage.tile([128, 512], BF, tag="ost")
                            nc.scalar.copy(ost[:], pso[n2][:])
                            nc.sync.dma_start(
                                out[b * XB:(b + 1) * XB,
                                    n2 * 512:(n2 + 1) * 512],
                                ost[:])
    nc.compile()
    return nc


def shard_inputs(x, W_qkv, b_qkv, pe, W_out, b_out):
    bf = ml_dtypes.bfloat16
    f8 = ml_dtypes.float8_e4m3
    scale = D ** -0.5
    x2 = np.asarray(x, np.float32).reshape(B * S, E)
    xT = np.ascontiguousarray(x2.T).astype(bf)
    Wq = (np.asarray(W_qkv[:, :E], np.float32) * scale).astype(bf)
    Wk = np.asarray(W_qkv[:, E:2 * E], np.float32).astype(bf)
    Wv = np.asarray(W_qkv[:, 2 * E:], np.float32).astype(bf)
    Wo = np.asarray(W_out, np.float32).astype(bf)
    bqv = (np.asarray(b_qkv[:E], np.float32) * scale).astype(bf).reshape(1, E)
    bkv = np.asarray(b_qkv[E:2 * E], np.float32).astype(bf).reshape(1, E)
    bvv = np.asarray(b_qkv[2 * E:], np.float32).astype(bf).reshape(1, E)
    bov = np.asarray(b_out, np.float32).astype(bf).reshape(1, E)

    pe32 = np.asarray(pe, np.float32)
    in_maps = []
    for c in range(NCORES):
        x0 = c * XB
        # pet[p, xh*64+c, y] = pe[x0 + xh*64 + p, y, c]
        pet_c = np.ascontiguousarray(
            pe32[x0:x0 + XB].transpose(0, 2, 1).reshape(2, XP, D, S)
            .transpose(1, 0, 2, 3)).reshape(XP, 128, S)
        cols = (np.arange(B)[:, None] * S + (x0 + np.arange(XB))[None, :]).ravel()
        xTo = np.ascontiguousarray(xT[:, cols])
        in_maps.append({
            "xTo": xTo,
            "wq": Wq, "wk": Wk, "wv": Wv, "wo": Wo,
            "pet": pet_c.astype(f8),
            "bq": bqv, "bk": bkv, "bv": bvv, "bo": bov,
        })
    return in_maps


def kernel(x, W_qkv, b_qkv, pe, W_out, b_out, _trace=False):
    global _compiled
    if _compiled is None:
        _compiled = build_kernel()
    nc = _compiled
    in_maps = shard_inputs(x, W_qkv, b_qkv, pe, W_out, b_out)
    res = run_bass_kernel_spmd(nc, in_maps, core_ids=list(range(NCORES)),
                               trace=_trace)
    outs = res.results
    full = np.empty((B, S, E), np.float32)
    for c in range(NCORES):
        full[:, c * XB:(c + 1) * XB, :] = (
            outs[c]["out"].astype(np.float32).reshape(B, XB, E))
    if _trace:
        kernel.last_exec_time_ns = res.exec_time_ns
        kernel.last_profile = res.profile_json
    return full
